# revision 3
# baseline (speedup 1.0000x reference)
"""Trainium2 Bass kernel for KVCacheHeavyHitters eviction update.

Full-input contract: kernel(**inputs) takes the unsharded inputs and returns
(new_k, new_v), each (1, 32, 8192, 128) float32.

Strategy: shard on the head axis across 8 NeuronCores (4 heads/core). Each
core, fully on-device:
  1. streams its att history slice ([8192, 4*128] probs/counts/valid),
     computes unimportance[l, h] = sum_w((p*c < 1) & valid) with the
     vector engine,
  2. score = unimp * L + l, per-head argmax via vector max/max_index,
  3. bulk-copies its k/v cache slice DRAM->DRAM,
  4. scatters k_val/v_val into the evicted row via indirect DMA.
Host only slices inputs per core and concatenates the per-core outputs.
"""
import numpy as np

B, H, L, D, W = 1, 32, 8192, 128, 128
NCORES = 8
HPC = H // NCORES        # heads per core = 4
FW = HPC * W             # att row width per core = 512
P = 128                  # SBUF partitions; l = p*NB + b
NB = L // P              # 64 b-columns per partition
SB = 4                   # b's per superblock
NSB = NB // SB           # 16 superblocks
CHUNK = HPC * L // NSB   # cache rows copied per superblock = 2048

_NC = None


def _build_nc():
    import concourse.bass as bass
    import concourse.bacc as bacc
    import concourse.mybir as mybir
    import concourse.tile as tile
    from concourse.tile import add_dep_helper

    f32 = mybir.dt.float32
    i32 = mybir.dt.int32
    u8 = mybir.dt.uint8
    u32 = mybir.dt.uint32

    nc = bacc.Bacc()
    att_p = nc.declare_dram_parameter("att_p", [L, FW], f32, isOutput=False)
    att_c = nc.declare_dram_parameter("att_c", [L, FW], i32, isOutput=False)
    att_v = nc.declare_dram_parameter("att_v", [L, FW], u8, isOutput=False)
    k_cache = nc.declare_dram_parameter("k_cache", [HPC * L, D], f32, isOutput=False)
    v_cache = nc.declare_dram_parameter("v_cache", [HPC * L, D], f32, isOutput=False)
    k_val = nc.declare_dram_parameter("k_val", [HPC, D], f32, isOutput=False)
    v_val = nc.declare_dram_parameter("v_val", [HPC, D], f32, isOutput=False)
    new_k = nc.declare_dram_parameter("new_k", [HPC * L, D], f32, isOutput=True)
    new_v = nc.declare_dram_parameter("new_v", [HPC, L, D], f32, isOutput=True)

    new_v2 = new_v[:].rearrange("h l d -> (h l) d")

    with tile.TileContext(nc) as tc:
        with tc.tile_pool(name="io", bufs=3) as io, \
             tc.tile_pool(name="acc", bufs=1) as acc:
            unimp = acc.tile([P, HPC, NB], f32)
            att_p_r = att_p[:].rearrange("(p nb) fw -> p nb fw", p=P)
            att_c_r = att_c[:].rearrange("(p nb) fw -> p nb fw", p=P)
            att_v_r = att_v[:].rearrange("(p nb) fw -> p nb fw", p=P)

            copy_k, copy_v = [], []
            for s in range(NSB):
                bs = slice(s * SB, (s + 1) * SB)
                pt = io.tile([P, SB, FW], f32, tag="pt")
                ct = io.tile([P, SB, FW], i32, tag="ct")
                vt = io.tile([P, SB, FW], u8, tag="vt")
                nc.sync.dma_start(out=pt[:], in_=att_p_r[:, bs, :])
                nc.sync.dma_start(out=ct[:], in_=att_c_r[:, bs, :])
                nc.sync.dma_start(out=vt[:], in_=att_v_r[:, bs, :])
                t = io.tile([P, SB, FW], f32, tag="t")
                nc.vector.tensor_tensor(out=t[:], in0=pt[:], in1=ct[:],
                                        op=mybir.AluOpType.mult)
                m = io.tile([P, SB, FW], f32, tag="m")
                nc.vector.tensor_tensor(out=m[:], in0=t[:], in1=vt[:],
                                        op=mybir.AluOpType.is_lt)
                for h in range(HPC):
                    nc.vector.tensor_reduce(
                        out=unimp[:, h, bs], in_=m[:, :, h * W:(h + 1) * W],
                        axis=mybir.AxisListType.X, op=mybir.AluOpType.add)

                rs = slice(s * CHUNK, (s + 1) * CHUNK)
                copy_k.append(nc.sync.dma_start(out=new_k[rs, :], in_=k_cache[rs, :]))
                copy_v.append(nc.sync.dma_start(out=new_v2[rs, :], in_=v_cache[rs, :]))

            # score = unimp * L + l, with l = p*NB + b
            l_mat = acc.tile([P, HPC, NB], i32)
            nc.gpsimd.iota(l_mat[:], pattern=[[0, HPC], [1, NB]], base=0,
                           channel_multiplier=NB)
            score = acc.tile([P, HPC, NB], f32)
            nc.vector.tensor_scalar(out=score[:], in0=unimp[:], scalar1=float(L),
                                    scalar2=None, op0=mybir.AluOpType.mult)
            nc.vector.tensor_tensor(out=score[:], in0=score[:], in1=l_mat[:],
                                    op=mybir.AluOpType.add)

            # flatten each head's [P, NB] score to [1, L] on partition h
            score_T = acc.tile([HPC, L], f32)
            for h in range(HPC):
                nc.sync.dma_start(out=score_T[h:h + 1, :], in_=score[:, h, :])

            maxv = acc.tile([HPC, 8], f32)
            idx = acc.tile([HPC, 8], u32)
            nc.vector.max(out=maxv[:], in_=score_T[:])
            nc.vector.max_index(out=idx[:], in_max=maxv[:], in_values=score_T[:])

            hoff = acc.tile([HPC, 1], u32)
            nc.gpsimd.iota(hoff[:], pattern=[[0, 1]], base=0, channel_multiplier=L)
            grow = acc.tile([HPC, 1], u32)
            nc.vector.tensor_tensor(out=grow[:], in0=idx[:, 0:1], in1=hoff[:],
                                    op=mybir.AluOpType.add)

            kval_sb = acc.tile([HPC, D], f32)
            vval_sb = acc.tile([HPC, D], f32)
            nc.sync.dma_start(out=kval_sb[:], in_=k_val[:])
            nc.sync.dma_start(out=vval_sb[:], in_=v_val[:])

            sk = nc.gpsimd.indirect_dma_start(
                out=new_k[:, :],
                out_offset=bass.IndirectOffsetOnAxis(ap=grow[:, :1], axis=0),
                in_=kval_sb[:, :], in_offset=None)
            sv = nc.gpsimd.indirect_dma_start(
                out=new_v2[:, :],
                out_offset=bass.IndirectOffsetOnAxis(ap=grow[:, :1], axis=0),
                in_=vval_sb[:, :], in_offset=None)
            for ci in copy_k:
                add_dep_helper(sk.ins, ci.ins, sync=True, reason="scatter after copy")
            for ci in copy_v:
                add_dep_helper(sv.ins, ci.ins, sync=True, reason="scatter after copy")
    nc.finalize()
    return nc


def _get_nc():
    global _NC
    if _NC is None:
        _NC = _build_nc()
    return _NC


def make_in_maps(k_cache, v_cache, k_val, v_val, att_probs, att_counts,
                 hist_valid, input_pos=None, pos=None):
    k_cache = np.asarray(k_cache)
    v_cache = np.asarray(v_cache)
    k_val = np.asarray(k_val)
    v_val = np.asarray(v_val)
    att_probs = np.asarray(att_probs)
    att_counts = np.asarray(att_counts)
    hist_valid = np.asarray(hist_valid).astype(np.uint8)
    in_maps = []
    for c in range(NCORES):
        hs = slice(c * HPC, (c + 1) * HPC)
        in_maps.append({
            "att_p": np.ascontiguousarray(att_probs[:, hs, :]).reshape(L, FW),
            "att_c": np.ascontiguousarray(att_counts[:, hs, :]).reshape(L, FW),
            "att_v": np.ascontiguousarray(hist_valid[:, hs, :]).reshape(L, FW),
            "k_cache": np.ascontiguousarray(k_cache[0, hs]).reshape(HPC * L, D),
            "v_cache": np.ascontiguousarray(v_cache[0, hs]).reshape(HPC * L, D),
            "k_val": np.ascontiguousarray(k_val[0, hs, 0, :]),
            "v_val": np.ascontiguousarray(v_val[0, hs, 0, :]),
        })
    return in_maps


def gather_outputs(results):
    new_k = np.concatenate(
        [results[c]["new_k"].reshape(1, HPC, L, D) for c in range(NCORES)], axis=1)
    new_v = np.concatenate(
        [results[c]["new_v"].reshape(1, HPC, L, D) for c in range(NCORES)], axis=1)
    return new_k, new_v


def kernel(**inputs):
    from concourse.bass_utils import run_bass_kernel_spmd
    nc = _get_nc()
    in_maps = make_in_maps(**inputs)
    res = run_bass_kernel_spmd(nc, in_maps, list(range(NCORES)))
    return gather_outputs(res.results)
